# revision 18
# baseline (speedup 1.0000x reference)
"""AttentiveProtoFusion kernel for 8 TRN2 NeuronCores.

Math (equivalent to reference, ~14x fewer FLOPs):
    q' = sent @ (Wq @ Wk^T) + bq @ Wk^T      [n, 768]   (folded host-side)
    scores[n,p] = sum_c proto[n,p,c] * q'[n,c]
    w = softmax(scores, axis=p);  ctx[n,c] = sum_p w[n,p] * proto[n,p,c]

Sharding: data-parallel over the 2048 tokens (B*S), 256/core, 2 blocks of
128 tokens (tokens on partitions). proto/sent/W staged host-side in fp16
(rel err ~2.8e-3 vs the 2e-2 gate; halves DMA to 12 MiB/core).

The kernel is limited by the shared SBUF access fabric, not any single
engine's ALUs: concurrent streaming engines slow each other ~2.5x
(measured). So the design minimises total SBUF port-seconds rather than
spreading multiplies across engines:
  DVE   : ALL 64 score passes, one fused scalar_tensor_tensor each
          (accum_out = score; the mandatory elementwise out goes to a
          scratch PSUM bank so no SBUF write port is burned); softmax
          frames (fixed Mhat = max(chunk0)+60, clamp +140, proven in the
          fp32 baseline) and sm = min(s,clamp)-Mhat.
  ACT   : ALL 64 pooling weights, built directly as bf16 DIAGONAL
          matrices dg = Exp(eyeNEG + sm_p) (eyeNEG = -60000 off-diagonal
          so exp -> 0); per-chunk exp of sm into fp32 expw for Z; q'
          copies; final ctx = U * (1/Z).
  PE    : q' projection + the whole pooling MAC as
          matmul(lhsT=diag(e_p) bf16, rhs=proto_p fp16) accumulating in
          PSUM fp32 (mixed 16-bit dtypes verified exact on HW; e spans
          up to e^80, safe in bf16).
  GPSIMD: idle - any work it does steals SBUF ports at a worse rate.
  DMA   : proto fp16 stream 12 MiB/core; weights split over the Sync and
          Scalar DGE queues so they load in parallel.
Per chunk of 8 protos the chain DVE scores -> DVE sm -> ACT exp+diags ->
PE MACs pipelines cleanly with no cross-engine rendezvous inside an
engine's own chunk work.
"""

import sys

for _p in ("/opt/trn_rl_repo", "/opt/pypackages"):
    if _p not in sys.path:
        sys.path.append(_p)

import numpy as np

B, S, P, D_SENT, D_CTX = 4, 512, 32, 1024, 768
N_CORES = 8
TOK = B * S                    # 2048
TPC = TOK // N_CORES           # 256 tokens per core
BLK = 128                      # tokens per block
NBLK = TPC // BLK              # 2
CH = 8                         # protos per chunk
NCH = P // CH                  # 4 chunks per block
EH = D_CTX // 2                # 384 = PSUM-bank-sized half
DS = D_SENT // 128             # 8 contraction chunks for the projection

_NC = None


def _build():
    import concourse.tile as tile
    from concourse import bacc, mybir

    f32 = mybir.dt.float32
    f16 = mybir.dt.float16
    bf16 = mybir.dt.bfloat16
    Alu = mybir.AluOpType
    Act = mybir.ActivationFunctionType
    X = mybir.AxisListType.X

    nc = bacc.Bacc("TRN2", target_bir_lowering=False)

    sentT_d = nc.dram_tensor("sentT", [128, DS, TPC], f16, kind="ExternalInput")
    proto_d = nc.dram_tensor(
        "proto", [NBLK, NCH, BLK, CH, D_CTX], f16, kind="ExternalInput"
    )
    w_d = nc.dram_tensor("w", [128, DS, D_CTX], f16, kind="ExternalInput")
    bp_d = nc.dram_tensor("bp", [1, D_CTX], f16, kind="ExternalInput")
    eye_d = nc.dram_tensor("eye", [128, 128], bf16, kind="ExternalInput")
    out_d = nc.dram_tensor("out", [TPC, D_CTX], f16, kind="ExternalOutput")

    with tile.TileContext(nc) as tc:
        with (
            tc.tile_pool(name="persist", bufs=1) as persist,
            tc.tile_pool(name="wpool", bufs=1) as wpool,
            tc.tile_pool(name="ppool", bufs=8) as ppool,
            tc.tile_pool(name="dpool", bufs=12) as dpool,
            tc.tile_pool(name="small", bufs=6) as small,
            tc.tile_pool(name="psum", bufs=4, space="PSUM") as psum,
            tc.tile_pool(name="jpsum", bufs=1, space="PSUM") as jpsum,
            tc.tile_pool(name="gpsum", bufs=1, space="PSUM") as gpsum,
        ):
            scores = persist.tile([128, NBLK, P], f32)
            sm = persist.tile([128, NBLK, P], f32)       # clamped, shifted
            expw = persist.tile([128, NBLK, P], f32)
            negM = persist.tile([128, NBLK, 1], f32)
            clampv = persist.tile([128, NBLK, 1], f32)
            qp_sb = persist.tile([128, NBLK, D_CTX], f16)
            out_sb = persist.tile([128, NBLK, D_CTX], f16)

            # ------------- weights (two DGE queues) + projection -------
            w_sb = wpool.tile([128, DS, D_CTX], f16)
            nc.sync.dma_start(out=w_sb[:], in_=w_d[:])
            sentT_sb = wpool.tile([128, DS, TPC], f16)
            nc.scalar.dma_start(out=sentT_sb[:], in_=sentT_d[:])
            bp_sb = wpool.tile([1, D_CTX], f16)
            nc.scalar.dma_start(out=bp_sb[:], in_=bp_d[:])
            eye_sb = wpool.tile([128, 128], bf16)
            nc.scalar.dma_start(out=eye_sb[:], in_=eye_d[:])
            eyeneg_sb = wpool.tile([128, 128], f32)
            nc.vector.tensor_scalar(
                eyeneg_sb[:], eye_sb[:], 60000.0, -60000.0,
                Alu.mult, Alu.add,
            )
            ones_sb = wpool.tile([1, 128], f16)
            nc.vector.memset(ones_sb[:], 1.0)

            for b in range(NBLK):
                for h in range(2):
                    pp = psum.tile([128, EH], f32, tag="ps")
                    for dd in range(DS):
                        nc.tensor.matmul(
                            pp[:],
                            sentT_sb[:, dd, b * BLK:(b + 1) * BLK],
                            w_sb[:, dd, h * EH:(h + 1) * EH],
                            start=(dd == 0),
                            stop=False,
                        )
                    nc.tensor.matmul(
                        pp[:],
                        ones_sb[0:1, :],
                        bp_sb[0:1, h * EH:(h + 1) * EH],
                        start=False,
                        stop=True,
                    )
                    nc.scalar.copy(
                        out=qp_sb[:, b, h * EH:(h + 1) * EH], in_=pp[:]
                    )

            # ---------------- online softmax-pooling ------------------
            jk = jpsum.tile([128, D_CTX], f32)           # stt scratch out
            Upsum = {}

            CHUNKS = {0: (8, 8, 8, 8), 1: (8, 12, 10, 2)}
            GPS_CNT = {(0, 0): 0, (0, 1): 0, (0, 2): 0, (0, 3): 0,
                       (1, 0): 0, (1, 1): 0, (1, 2): 0, (1, 3): 0}
            gs = gpsum.tile([128, D_CTX], f32)       # GPSIMD product scratch
            tiles = {}
            cks = []
            for b in range(NBLK):
                sizes = CHUNKS[b]
                offs = [sum(sizes[:i]) for i in range(len(sizes))]
                for c in range(len(sizes)):
                    cks.append((b, c, offs[c], sizes[c], GPS_CNT[(b, c)]))

            def tsrc(b, p):
                return tiles[b][p // CH][:, p % CH, :]

            def emit_dma(b):
                tl = []
                for t in range(NCH):
                    T = ppool.tile([128, CH, D_CTX], f16, tag="T")
                    nc.sync.dma_start(out=T[:], in_=proto_d[b, t])
                    tl.append(T)
                tiles[b] = tl

            def emit_scores(ck):
                b, c, p0, nch, ng = ck
                for j in range(ng):              # GPSIMD products first
                    p = p0 + j
                    nc.gpsimd.tensor_tensor(
                        out=gs[:], in0=tsrc(b, p), in1=qp_sb[:, b, :],
                        op=Alu.mult,
                    )
                for j in range(ng, nch):
                    p = p0 + j
                    nc.vector.scalar_tensor_tensor(
                        out=jk[:],
                        in0=tsrc(b, p),
                        scalar=0.0,
                        in1=qp_sb[:, b, :],
                        op0=Alu.bypass,
                        op1=Alu.mult,
                        accum_out=scores[:, b, p:p + 1],
                    )

            def emit_accums(ck):
                b, c, p0, nch, ng = ck
                for j in range(ng):
                    p = p0 + j
                    nc.scalar.activation(
                        out=gs[:], in_=gs[:], func=Act.Copy,
                        accum_out=scores[:, b, p:p + 1],
                    )

            def emit_post(ck):
                b, c, p0, nch, ng = ck
                if c == 0:
                    m8 = small.tile([128, 1], f32, tag="m8")
                    nc.vector.tensor_reduce(
                        out=m8[:], in_=scores[:, b, p0:p0 + nch],
                        axis=X, op=Alu.max,
                    )
                    nc.vector.tensor_scalar(
                        negM[:, b, :], m8[:], -1.0, -60.0, Alu.mult, Alu.add,
                    )
                    nc.vector.tensor_scalar(
                        clampv[:, b, :], m8[:], 1.0, 140.0, Alu.mult, Alu.add,
                    )
                nc.vector.tensor_scalar(
                    sm[:, b, p0:p0 + nch], scores[:, b, p0:p0 + nch],
                    clampv[:, b, :], negM[:, b, :], Alu.min, Alu.add,
                )
                nc.scalar.activation(
                    out=expw[:, b, p0:p0 + nch], in_=sm[:, b, p0:p0 + nch],
                    func=Act.Exp, bias=0.0, scale=1.0,
                )
                if c == 0:
                    ulo = psum.tile([128, EH], f32, tag="ps")
                    uhi = psum.tile([128, EH], f32, tag="ps")
                    Upsum[b] = (ulo, uhi)
                ulo, uhi = Upsum[b]
                for j in range(nch):
                    p = p0 + j
                    dg = dpool.tile([128, 128], bf16, tag="dg")
                    nc.scalar.activation(
                        out=dg[:], in_=eyeneg_sb[:], func=Act.Exp,
                        bias=sm[:, b, p:p + 1], scale=1.0,
                    )
                    nc.tensor.matmul(
                        ulo[:], dg[:], tsrc(b, p)[:, 0:EH],
                        start=(p == 0), stop=(p == P - 1),
                    )
                    nc.tensor.matmul(
                        uhi[:], dg[:], tsrc(b, p)[:, EH:],
                        start=(p == 0), stop=(p == P - 1),
                    )
                if c == NCH - 1:
                    z = small.tile([128, 1], f32, tag="z")
                    nc.vector.tensor_reduce(
                        out=z[:], in_=expw[:, b, :], axis=X, op=Alu.add,
                    )
                    rinv = small.tile([128, 1], f32, tag="rinv")
                    nc.vector.reciprocal(out=rinv[:], in_=z[:])
                    nc.scalar.activation(
                        out=out_sb[:, b, 0:EH], in_=ulo[:], func=Act.Copy,
                        scale=rinv[:],
                    )
                    nc.scalar.activation(
                        out=out_sb[:, b, EH:], in_=uhi[:], func=Act.Copy,
                        scale=rinv[:],
                    )
                    nc.sync.dma_start(
                        out=out_d[b * BLK:(b + 1) * BLK, :],
                        in_=out_sb[:, b, :],
                    )

            emit_dma(0)
            emit_dma(1)
            pending = None
            for ck in cks:
                emit_scores(ck)
                if pending is not None:
                    emit_post(pending)
                    pending = None
                if ck[4] == 0:
                    emit_post(ck)
                else:
                    pending = ck
                emit_accums(ck)
            if pending is not None:
                emit_post(pending)

    nc.compile()
    return nc


def _get_nc():
    global _NC
    if _NC is None:
        _NC = _build()
    return _NC


def _make_in_maps(sent_vecs, proto_vecs, Wq, bq, Wk):
    import ml_dtypes

    f16 = np.float16
    sent = np.asarray(sent_vecs, dtype=np.float32).reshape(TOK, D_SENT)
    sentT = sent.T.astype(f16)                                # [D_SENT, TOK]
    proto16 = np.asarray(proto_vecs, dtype=np.float32).reshape(
        TOK, P, D_CTX).astype(f16)
    wq = np.asarray(Wq, dtype=np.float32)
    bq = np.asarray(bq, dtype=np.float32).reshape(1, D_CTX)
    wk = np.asarray(Wk, dtype=np.float32)
    # pre-arranged for contiguous DMA: [(dd p), x] -> [p, dd, x]
    w = np.ascontiguousarray(
        (wq @ wk.T).astype(f16).reshape(DS, 128, D_CTX).transpose(1, 0, 2)
    )
    bp = np.ascontiguousarray((bq @ wk.T).astype(f16))
    eye = np.ascontiguousarray(np.eye(128, dtype=ml_dtypes.bfloat16))
    in_maps = []
    for i in range(N_CORES):
        sl = slice(i * TPC, (i + 1) * TPC)
        st = np.ascontiguousarray(
            sentT[:, sl].reshape(DS, 128, TPC).transpose(1, 0, 2)
        )
        pr = np.ascontiguousarray(
            proto16[sl].reshape(NBLK, BLK, NCH, CH, D_CTX)
            .transpose(0, 2, 1, 3, 4)
        )
        in_maps.append(
            {"sentT": st, "proto": pr, "w": w, "bp": bp, "eye": eye}
        )
    return in_maps


def _ensure_ntff_hook():
    """The agent image's antenv lacks axon_hooks; shim it so trace=True
    can capture NTFF profiles via the libaxon ctypes path."""
    try:
        from antenv.axon_hooks import get_axon_ntff_profile_hook  # noqa: F401
        return
    except ImportError:
        pass
    import types

    import antenv
    from trn_agent_boot.trn_boot import _ntff_profile_via_ctypes

    mod = types.ModuleType("antenv.axon_hooks")
    mod._hook = _ntff_profile_via_ctypes("/opt/axon/libaxon_pjrt.so")
    mod.get_axon_ntff_profile_hook = lambda: mod._hook
    mod.set_axon_ntff_profile_hook = lambda h: setattr(mod, "_hook", h)
    sys.modules["antenv.axon_hooks"] = mod
    antenv.axon_hooks = mod


def run(sent_vecs, proto_vecs, Wq, bq, Wk, bk=None, trace=False, **kw):
    """Returns (out[4,512,768] float32, BassKernelResults)."""
    from concourse.bass_utils import run_bass_kernel_spmd

    if trace:
        _ensure_ntff_hook()
    nc = _get_nc()
    in_maps = _make_in_maps(sent_vecs, proto_vecs, Wq, bq, Wk)
    res = run_bass_kernel_spmd(
        nc, in_maps, core_ids=list(range(N_CORES)), trace=trace
    )
    outs = [np.asarray(res.results[i]["out"]) for i in range(N_CORES)]
    full = np.concatenate(outs, axis=0).reshape(B, S, D_CTX).astype(np.float32)
    return full, res


def kernel(sent_vecs, proto_vecs, Wq, bq, Wk, bk=None, **kw):
    out, _ = run(sent_vecs, proto_vecs, Wq, bq, Wk, bk)
    return out


if __name__ == "__main__":
    nc = _get_nc()
    print("build + compile OK")


# revision 19
# speedup vs baseline: 1.0001x; 1.0001x over previous
"""AttentiveProtoFusion kernel for 8 TRN2 NeuronCores.

Math (equivalent to reference, ~14x fewer FLOPs):
    q' = sent @ (Wq @ Wk^T) + bq @ Wk^T      [n, 768]   (folded host-side)
    scores[n,p] = sum_c proto[n,p,c] * q'[n,c]
    w = softmax(scores, axis=p);  ctx[n,c] = sum_p w[n,p] * proto[n,p,c]

Sharding: data-parallel over the 2048 tokens (B*S), 256/core, 2 blocks of
128 tokens (tokens on partitions). proto/sent/W staged host-side in fp16
(rel err ~2.8e-3 vs the 2e-2 gate; halves DMA to 12 MiB/core).

The kernel is limited by the shared SBUF access fabric, not any single
engine's ALUs: concurrent streaming engines slow each other ~2.5x
(measured). So the design minimises total SBUF port-seconds rather than
spreading multiplies across engines:
  DVE   : ALL 64 score passes, one fused scalar_tensor_tensor each
          (accum_out = score; the mandatory elementwise out goes to a
          scratch PSUM bank so no SBUF write port is burned); softmax
          frames (fixed Mhat = max(chunk0)+60, clamp +140, proven in the
          fp32 baseline) and sm = min(s,clamp)-Mhat.
  ACT   : ALL 64 pooling weights, built directly as bf16 DIAGONAL
          matrices dg = Exp(eyeNEG + sm_p) (eyeNEG = -60000 off-diagonal
          so exp -> 0); per-chunk exp of sm into fp32 expw for Z; q'
          copies; final ctx = U * (1/Z).
  PE    : q' projection + the whole pooling MAC as
          matmul(lhsT=diag(e_p) bf16, rhs=proto_p fp16) accumulating in
          PSUM fp32 (mixed 16-bit dtypes verified exact on HW; e spans
          up to e^80, safe in bf16).
  GPSIMD: idle - any work it does steals SBUF ports at a worse rate.
  DMA   : proto fp16 stream 12 MiB/core; weights split over the Sync and
          Scalar DGE queues so they load in parallel.
Per chunk of 8 protos the chain DVE scores -> DVE sm -> ACT exp+diags ->
PE MACs pipelines cleanly with no cross-engine rendezvous inside an
engine's own chunk work.
"""

import sys

for _p in ("/opt/trn_rl_repo", "/opt/pypackages"):
    if _p not in sys.path:
        sys.path.append(_p)

import numpy as np

B, S, P, D_SENT, D_CTX = 4, 512, 32, 1024, 768
N_CORES = 8
TOK = B * S                    # 2048
TPC = TOK // N_CORES           # 256 tokens per core
BLK = 128                      # tokens per block
NBLK = TPC // BLK              # 2
CH = 8                         # protos per chunk
NCH = P // CH                  # 4 chunks per block
EH = D_CTX // 2                # 384 = PSUM-bank-sized half
DS = D_SENT // 128             # 8 contraction chunks for the projection

_NC = None


def _build():
    import concourse.tile as tile
    from concourse import bacc, mybir

    f32 = mybir.dt.float32
    f16 = mybir.dt.float16
    bf16 = mybir.dt.bfloat16
    Alu = mybir.AluOpType
    Act = mybir.ActivationFunctionType
    X = mybir.AxisListType.X

    nc = bacc.Bacc("TRN2", target_bir_lowering=False)

    sentT_d = nc.dram_tensor("sentT", [128, DS, TPC], f16, kind="ExternalInput")
    proto_d = nc.dram_tensor(
        "proto", [NBLK, NCH, BLK, CH, D_CTX], f16, kind="ExternalInput"
    )
    w_d = nc.dram_tensor("w", [128, DS, D_CTX], f16, kind="ExternalInput")
    bp_d = nc.dram_tensor("bp", [1, D_CTX], f16, kind="ExternalInput")
    eye_d = nc.dram_tensor("eye", [128, 128], bf16, kind="ExternalInput")
    out_d = nc.dram_tensor("out", [TPC, D_CTX], f16, kind="ExternalOutput")

    with tile.TileContext(nc) as tc:
        with (
            tc.tile_pool(name="persist", bufs=1) as persist,
            tc.tile_pool(name="wpool", bufs=1) as wpool,
            tc.tile_pool(name="ppool", bufs=8) as ppool,
            tc.tile_pool(name="dpool", bufs=12) as dpool,
            tc.tile_pool(name="small", bufs=6) as small,
            tc.tile_pool(name="psum", bufs=4, space="PSUM") as psum,
            tc.tile_pool(name="jpsum", bufs=1, space="PSUM") as jpsum,
            tc.tile_pool(name="gpsum", bufs=1, space="PSUM") as gpsum,
        ):
            scores = persist.tile([128, NBLK, P], f32)
            sm = persist.tile([128, NBLK, P], f32)       # clamped, shifted
            expw = persist.tile([128, NBLK, P], f32)
            negM = persist.tile([128, NBLK, 1], f32)
            clampv = persist.tile([128, NBLK, 1], f32)
            qp_sb = persist.tile([128, NBLK, D_CTX], f16)
            out_sb = persist.tile([128, NBLK, D_CTX], f16)

            # ------------- weights (two DGE queues) + projection -------
            w_sb = wpool.tile([128, DS, D_CTX], f16)
            nc.sync.dma_start(out=w_sb[:], in_=w_d[:])
            sentT_sb = wpool.tile([128, DS, TPC], f16)
            nc.scalar.dma_start(out=sentT_sb[:], in_=sentT_d[:])
            bp_sb = wpool.tile([1, D_CTX], f16)
            nc.scalar.dma_start(out=bp_sb[:], in_=bp_d[:])
            eye_sb = wpool.tile([128, 128], bf16)
            nc.scalar.dma_start(out=eye_sb[:], in_=eye_d[:])
            eyeneg_sb = wpool.tile([128, 128], f32)
            nc.vector.tensor_scalar(
                eyeneg_sb[:], eye_sb[:], 60000.0, -60000.0,
                Alu.mult, Alu.add,
            )
            ones_sb = wpool.tile([1, 128], f16)
            nc.vector.memset(ones_sb[:], 1.0)

            for b in range(NBLK):
                for h in range(2):
                    pp = psum.tile([128, EH], f32, tag="ps")
                    for dd in range(DS):
                        nc.tensor.matmul(
                            pp[:],
                            sentT_sb[:, dd, b * BLK:(b + 1) * BLK],
                            w_sb[:, dd, h * EH:(h + 1) * EH],
                            start=(dd == 0),
                            stop=False,
                        )
                    nc.tensor.matmul(
                        pp[:],
                        ones_sb[0:1, :],
                        bp_sb[0:1, h * EH:(h + 1) * EH],
                        start=False,
                        stop=True,
                    )
                    nc.scalar.copy(
                        out=qp_sb[:, b, h * EH:(h + 1) * EH], in_=pp[:]
                    )

            # ---------------- online softmax-pooling ------------------
            jk = jpsum.tile([128, D_CTX], f32)           # stt scratch out
            Upsum = {}

            CHUNKS = {0: (8, 8, 8, 8), 1: (8, 8, 8, 8)}
            GPS_CNT = {(0, 0): 0, (0, 1): 0, (0, 2): 0, (0, 3): 0,
                       (1, 0): 0, (1, 1): 0, (1, 2): 0, (1, 3): 0}
            gs = gpsum.tile([128, D_CTX], f32)       # GPSIMD product scratch
            tiles = {}
            cks = []
            for b in range(NBLK):
                sizes = CHUNKS[b]
                offs = [sum(sizes[:i]) for i in range(len(sizes))]
                for c in range(len(sizes)):
                    cks.append((b, c, offs[c], sizes[c], GPS_CNT[(b, c)]))

            def tsrc(b, p):
                return tiles[b][p // CH][:, p % CH, :]

            def emit_dma(b):
                tl = []
                for t in range(NCH):
                    T = ppool.tile([128, CH, D_CTX], f16, tag="T")
                    nc.sync.dma_start(out=T[:], in_=proto_d[b, t])
                    tl.append(T)
                tiles[b] = tl

            def emit_scores(ck):
                b, c, p0, nch, ng = ck
                for j in range(ng):              # GPSIMD products first
                    p = p0 + j
                    nc.gpsimd.tensor_tensor(
                        out=gs[:], in0=tsrc(b, p), in1=qp_sb[:, b, :],
                        op=Alu.mult,
                    )
                for j in range(ng, nch):
                    p = p0 + j
                    nc.vector.scalar_tensor_tensor(
                        out=jk[:],
                        in0=tsrc(b, p),
                        scalar=0.0,
                        in1=qp_sb[:, b, :],
                        op0=Alu.bypass,
                        op1=Alu.mult,
                        accum_out=scores[:, b, p:p + 1],
                    )

            def emit_accums(ck):
                b, c, p0, nch, ng = ck
                for j in range(ng):
                    p = p0 + j
                    nc.scalar.activation(
                        out=gs[:], in_=gs[:], func=Act.Copy,
                        accum_out=scores[:, b, p:p + 1],
                    )

            def emit_post(ck):
                b, c, p0, nch, ng = ck
                if c == 0:
                    m8 = small.tile([128, 1], f32, tag="m8")
                    nc.vector.tensor_reduce(
                        out=m8[:], in_=scores[:, b, p0:p0 + nch],
                        axis=X, op=Alu.max,
                    )
                    nc.vector.tensor_scalar(
                        negM[:, b, :], m8[:], -1.0, -60.0, Alu.mult, Alu.add,
                    )
                    nc.vector.tensor_scalar(
                        clampv[:, b, :], m8[:], 1.0, 140.0, Alu.mult, Alu.add,
                    )
                nc.vector.tensor_scalar(
                    sm[:, b, p0:p0 + nch], scores[:, b, p0:p0 + nch],
                    clampv[:, b, :], negM[:, b, :], Alu.min, Alu.add,
                )
                nc.scalar.activation(
                    out=expw[:, b, p0:p0 + nch], in_=sm[:, b, p0:p0 + nch],
                    func=Act.Exp, bias=0.0, scale=1.0,
                )
                if c == 0:
                    ulo = psum.tile([128, EH], f32, tag="ps")
                    uhi = psum.tile([128, EH], f32, tag="ps")
                    Upsum[b] = (ulo, uhi)
                ulo, uhi = Upsum[b]
                for j in range(nch):
                    p = p0 + j
                    dg = dpool.tile([128, 128], bf16, tag="dg")
                    nc.scalar.activation(
                        out=dg[:], in_=eyeneg_sb[:], func=Act.Exp,
                        bias=sm[:, b, p:p + 1], scale=1.0,
                    )
                    nc.tensor.matmul(
                        ulo[:], dg[:], tsrc(b, p)[:, 0:EH],
                        start=(p == 0), stop=(p == P - 1),
                    )
                    nc.tensor.matmul(
                        uhi[:], dg[:], tsrc(b, p)[:, EH:],
                        start=(p == 0), stop=(p == P - 1),
                    )
                if c == NCH - 1:
                    z = small.tile([128, 1], f32, tag="z")
                    nc.vector.tensor_reduce(
                        out=z[:], in_=expw[:, b, :], axis=X, op=Alu.add,
                    )
                    rinv = small.tile([128, 1], f32, tag="rinv")
                    nc.vector.reciprocal(out=rinv[:], in_=z[:])
                    nc.scalar.activation(
                        out=out_sb[:, b, 0:EH], in_=ulo[:], func=Act.Copy,
                        scale=rinv[:],
                    )
                    nc.scalar.activation(
                        out=out_sb[:, b, EH:], in_=uhi[:], func=Act.Copy,
                        scale=rinv[:],
                    )
                    nc.sync.dma_start(
                        out=out_d[b * BLK:(b + 1) * BLK, :],
                        in_=out_sb[:, b, :],
                    )

            emit_dma(0)
            emit_dma(1)
            pending = None
            for ck in cks:
                emit_scores(ck)
                if pending is not None:
                    emit_post(pending)
                    pending = None
                if ck[4] == 0:
                    emit_post(ck)
                else:
                    pending = ck
                emit_accums(ck)
            if pending is not None:
                emit_post(pending)

    nc.compile()
    return nc


def _get_nc():
    global _NC
    if _NC is None:
        _NC = _build()
    return _NC


def _make_in_maps(sent_vecs, proto_vecs, Wq, bq, Wk):
    import ml_dtypes

    f16 = np.float16
    sent = np.asarray(sent_vecs, dtype=np.float32).reshape(TOK, D_SENT)
    sentT = sent.T.astype(f16)                                # [D_SENT, TOK]
    proto16 = np.asarray(proto_vecs, dtype=np.float32).reshape(
        TOK, P, D_CTX).astype(f16)
    wq = np.asarray(Wq, dtype=np.float32)
    bq = np.asarray(bq, dtype=np.float32).reshape(1, D_CTX)
    wk = np.asarray(Wk, dtype=np.float32)
    # pre-arranged for contiguous DMA: [(dd p), x] -> [p, dd, x]
    w = np.ascontiguousarray(
        (wq @ wk.T).astype(f16).reshape(DS, 128, D_CTX).transpose(1, 0, 2)
    )
    bp = np.ascontiguousarray((bq @ wk.T).astype(f16))
    eye = np.ascontiguousarray(np.eye(128, dtype=ml_dtypes.bfloat16))
    in_maps = []
    for i in range(N_CORES):
        sl = slice(i * TPC, (i + 1) * TPC)
        st = np.ascontiguousarray(
            sentT[:, sl].reshape(DS, 128, TPC).transpose(1, 0, 2)
        )
        pr = np.ascontiguousarray(
            proto16[sl].reshape(NBLK, BLK, NCH, CH, D_CTX)
            .transpose(0, 2, 1, 3, 4)
        )
        in_maps.append(
            {"sentT": st, "proto": pr, "w": w, "bp": bp, "eye": eye}
        )
    return in_maps


def _ensure_ntff_hook():
    """The agent image's antenv lacks axon_hooks; shim it so trace=True
    can capture NTFF profiles via the libaxon ctypes path."""
    try:
        from antenv.axon_hooks import get_axon_ntff_profile_hook  # noqa: F401
        return
    except ImportError:
        pass
    import types

    import antenv
    from trn_agent_boot.trn_boot import _ntff_profile_via_ctypes

    mod = types.ModuleType("antenv.axon_hooks")
    mod._hook = _ntff_profile_via_ctypes("/opt/axon/libaxon_pjrt.so")
    mod.get_axon_ntff_profile_hook = lambda: mod._hook
    mod.set_axon_ntff_profile_hook = lambda h: setattr(mod, "_hook", h)
    sys.modules["antenv.axon_hooks"] = mod
    antenv.axon_hooks = mod


def run(sent_vecs, proto_vecs, Wq, bq, Wk, bk=None, trace=False, **kw):
    """Returns (out[4,512,768] float32, BassKernelResults)."""
    from concourse.bass_utils import run_bass_kernel_spmd

    if trace:
        _ensure_ntff_hook()
    nc = _get_nc()
    in_maps = _make_in_maps(sent_vecs, proto_vecs, Wq, bq, Wk)
    res = run_bass_kernel_spmd(
        nc, in_maps, core_ids=list(range(N_CORES)), trace=trace
    )
    outs = [np.asarray(res.results[i]["out"]) for i in range(N_CORES)]
    full = np.concatenate(outs, axis=0).reshape(B, S, D_CTX).astype(np.float32)
    return full, res


def kernel(sent_vecs, proto_vecs, Wq, bq, Wk, bk=None, **kw):
    out, _ = run(sent_vecs, proto_vecs, Wq, bq, Wk, bk)
    return out


if __name__ == "__main__":
    nc = _get_nc()
    print("build + compile OK")
